# revision 1
# baseline (speedup 1.0000x reference)
"""BRD4KANModel Trainium2 kernel.

Data-parallel over batch across 8 NeuronCores (512 rows each, weights
replicated). All weights are pre-transposed / pre-tiled / bf16-cast on the
host into matmul-ready lhsT layout ([in-feature partitions, out-feature
free]), with the spline scaler and the truncated-power scale lam folded in.
The device therefore runs ONLY real matmuls on the PE (no transposes), the
B-spline bases on ACT+DVE, and PSUM evacuations.

B-spline bases via truncated powers: with h' = lam*h and c_m = lam*g_m,
z_m = relu(h' - c_m), the 6 cubic bases are the 4th forward differences
b_c = z³_c - 4z³_{c+1} + 6z³_{c+2} - 4z³_{c+3} + z³_{c+4}, computed as a
grouped 24-op DVE cascade per 128-feature tile. Each z³ comes either from
ACT relu+square plus a DVE multiply, or entirely on ACT as
exp(3*ln(relu)) — the per-unit split (na) balances the two engines.
GPSIMD gets nothing: it shares an SBUF port with the DVE, so offloading
elementwise work there is a wash.

Layer matmuls are split into 4 k-quarters (i-tiles 0-3, 4-7, 8-11, 12-15)
so only ~2 quarters of bases tiles are ever live (SBUF), with an f32 SBUF
accumulator carrying partial sums between quarters. Bases for each sweep
are produced one sweep ahead, overlapping DVE/ACT work with PE matmuls.
ACT table-sets (sigmoid / silu / ln+exp) are kept temporally disjoint:
silu batches are gated behind a zero-tile written by a late DVE op, and
boundary bases units avoid ln/exp, otherwise the list scheduler thrashes
table loads (~2.7us each).

This walrus build accepts only ONE semaphore wait per instruction, while
Tile's scheduler attaches several; _split_waits() post-processes the BIR
JSON, hoisting excess waits onto NoOps inserted just before each
instruction on the same engine.
"""

import json

import numpy as np
import ml_dtypes

import concourse.bass as bass
import concourse.mybir as mybir
import concourse.tile as tile

F32 = mybir.dt.float32
BF16 = mybir.dt.bfloat16
AF = mybir.ActivationFunctionType
OP = mybir.AluOpType

N_CORES = 8
BATCH = 4096
B = BATCH // N_CORES  # 512 per core
D = 2048
WIDTHS = [2048, 2048, 1024]
COEFF = 6
GRID_SIZE = 3
SPLINE_ORDER = 3
HSTEP = 2.0 / GRID_SIZE
GRID = [m * HSTEP - 1.0 - SPLINE_ORDER * HSTEP
        for m in range(GRID_SIZE + 2 * SPLINE_ORDER + 1)]  # 10 knots, -3..3
LAM = float((6.0 * HSTEP ** 3) ** (-1.0 / 3.0))
NK = 10          # truncated-power knots
IT = 16          # 2048/128 input tiles per layer
NQ = 4           # k-quarters
KQ = IT // NQ    # i-tiles per quarter
SW = 512 + KQ * COEFF * 128  # combined strip width per (o,q): base + spline


def _split_waits(bir_bytes: bytes, keep: int = 1) -> bytes:
    d = json.loads(bir_bytes)
    for f in d["functions"]:
        for bb in f["blocks"]:
            new_insts = []
            for inst in bb["instructions"]:
                si = inst.get("sync_info")
                waits = (si or {}).get("on_wait") or []
                if len(waits) > keep:
                    extra = waits[:-keep]
                    inst["sync_info"]["on_wait"] = waits[-keep:]
                    for ci in range(0, len(extra), keep):
                        new_insts.append({
                            "name": f"{inst['name']}-w{ci}",
                            "opcode": "NoOp",
                            "engine": inst["engine"],
                            "ins": [],
                            "outs": [],
                            "debug": inst.get("debug"),
                            "sync_info": {"on_update": [],
                                          "on_wait": extra[ci:ci + keep]},
                        })
                new_insts.append(inst)
            bb["instructions"] = new_insts
    return json.dumps(d).encode()


def _patch_json(nc):
    orig = nc.to_json_bytes

    def patched():
        return _split_waits(orig())

    nc.to_json_bytes = patched
    return nc


def build():
    nc = bass.Bass()
    xT = nc.dram_tensor("xT", [D, B], BF16, kind="ExternalInput")
    wm = nc.dram_tensor("wm", [32 * 128, D], BF16, kind="ExternalInput")
    mbg = nc.dram_tensor("mbg", [128, 16], F32, kind="ExternalInput")
    mbv = nc.dram_tensor("mbv", [128, 16], F32, kind="ExternalInput")
    ws_d = []
    for l, fo in enumerate(WIDTHS):
        ot = fo // 128
        ws_d.append(nc.dram_tensor(f"ws{l}", [ot * NQ * 128, SW], BF16,
                                   kind="ExternalInput"))
    wh = nc.dram_tensor("wh", [128, 16], BF16, kind="ExternalInput")
    hb = nc.dram_tensor("hb", [2, 1], F32, kind="ExternalInput")
    out = nc.dram_tensor("out", [2, B], F32, kind="ExternalOutput")

    with tile.TileContext(nc) as tc:
        with tc.tile_pool(name="consts", bufs=1) as consts, \
             tc.tile_pool(name="wmp", bufs=2) as wmp, \
             tc.tile_pool(name="wsp", bufs=3) as wsp, \
             tc.tile_pool(name="hp", bufs=33) as hp, \
             tc.tile_pool(name="silup", bufs=19) as silup, \
             tc.tile_pool(name="basp", bufs=49) as basp, \
             tc.tile_pool(name="zp", bufs=10) as zp, \
             tc.tile_pool(name="rtp", bufs=5) as rtp, \
             tc.tile_pool(name="qp", bufs=6) as qp, \
             tc.tile_pool(name="psA", bufs=7, space="PSUM") as psA, \
             tc.tile_pool(name="psH", bufs=1, space="PSUM") as psH:

            # ---- constants ----
            cm = consts.tile([128, NK], F32, tag="cm")
            for m in range(NK):
                nc.vector.memset(cm[:, m:m + 1], float(-LAM * GRID[m]))
            mbg_sb = consts.tile([128, 16], F32, tag="mbg")
            nc.scalar.dma_start(mbg_sb, mbg[:])
            mbv_sb = consts.tile([128, 16], F32, tag="mbv")
            nc.scalar.dma_start(mbv_sb, mbv[:])
            wh_sb = consts.tile([128, 16], BF16, tag="wh")
            nc.scalar.dma_start(wh_sb, wh[:])
            hb_sb = consts.tile([2, 1], F32, tag="hb")
            nc.scalar.dma_start(hb_sb, hb[:])

            # ---- x^T tiles (host pre-transposed; share bas slots) ----
            xb = []
            for i in range(IT):
                t = basp.tile([128, B], BF16, tag="bas", name=f"x{i}")
                nc.scalar.dma_start(t, xT[i * 128:(i + 1) * 128, :])
                xb.append(t)

            # warmup: keep the PE busy through the DMA startup so the HAM
            # clock-gate reaches 8/8 before the real matmuls begin
            wz = consts.tile([128, 128], BF16, tag="wz")
            nc.vector.memset(wz, 0.0)
            accw = psH.tile([128, B], F32, tag="acch", name="warm")
            for k in range(60):
                nc.tensor.matmul(accw[:, 0:128], wz, wz, start=(k == 0),
                                 stop=(k == 59))

            silu_t = {}
            bas_t = {}

            def emit_silu(l, i, h_t, bias=0.0):
                # bias is always zero-valued; passing a gate tile written by
                # a late DVE op delays readiness so the list scheduler can't
                # pull Silu (its own ACT table-set) into an earlier window
                st = silup.tile([128, B], BF16, tag="silu",
                                name=f"silu{l}_{i}")
                nc.scalar.activation(st, h_t, AF.Silu, scale=1.0 / LAM,
                                     bias=bias)
                silu_t[(l, i)] = st

            z3_pend = {}

            def emit_A_z(l, i, h_t, na):
                """z^3 tiles for layer-l input tile i.

                na of the 10 cubes go through ACT (exp(3*ln(relu))), the
                rest use ACT relu+square plus a DVE multiply. Splitting
                balances the two engines (the GPSIMD SBUF port contention
                makes POOL offload a wash, so it gets nothing).
                """
                z3 = []
                for m in range(NK):
                    r = rtp.tile([128, B], F32, tag="rt", name=f"r{m}")
                    nc.scalar.activation(r, h_t, AF.Relu,
                                         bias=cm[:, m:m + 1])
                    z = zp.tile([128, B], F32, tag="z", name=f"z3_{m}")
                    if m < na:
                        lnr = rtp.tile([128, B], F32, tag="rt",
                                       name=f"lnr{m}")
                        nc.scalar.activation(lnr, r, AF.Ln)
                        nc.scalar.activation(z, lnr, AF.Exp, scale=3.0)
                    else:
                        t2 = rtp.tile([128, B], F32, tag="rt",
                                      name=f"t2{m}")
                        nc.scalar.activation(t2, h_t, AF.Square,
                                             bias=cm[:, m:m + 1])
                        nc.vector.tensor_tensor(z, r, t2, OP.mult)
                    z3.append(z)
                z3_pend[(l, i)] = z3
            def emit_A_casc(l, i):
                z3 = z3_pend.pop((l, i))
                for c in range(COEFF):
                    q = qp.tile([128, B], F32, tag="q", name=f"q{c}")
                    nc.vector.tensor_tensor(q, z3[c], z3[c + 4], OP.add)
                    r2 = qp.tile([128, B], F32, tag="q", name=f"r2{c}")
                    nc.vector.tensor_tensor(r2, z3[c + 1], z3[c + 3], OP.add)
                    nc.vector.scalar_tensor_tensor(q, r2, -4.0, q,
                                                   OP.mult, OP.add)
                    bt = basp.tile([128, B], BF16, tag="bas",
                                   name=f"bas{l}_{i}_{c}")
                    nc.vector.scalar_tensor_tensor(bt, z3[c + 2], 6.0, q,
                                                   OP.mult, OP.add)
                    bas_t[(l, i, c)] = bt

            def emit_A(l, i, h_t, na):
                emit_A_z(l, i, h_t, na)
                emit_A_casc(l, i)

            # ---- multiplicative layer ----
            h_cur = []
            for j in range(IT):
                wg = wmp.tile([128, D], BF16, tag="wm", name=f"wg{j}")
                nc.sync.dma_start(wg, wm[j * 128:(j + 1) * 128, :])
                accg = psA.tile([128, B], F32, tag="acc")
                for k in range(IT):
                    nc.tensor.matmul(accg, wg[:, k * 128:(k + 1) * 128],
                                     xb[k], start=(k == 0),
                                     stop=(k == IT - 1))
                sig = qp.tile([128, B], F32, tag="q", name=f"sig{j}")
                nc.scalar.activation(sig, accg, AF.Sigmoid,
                                     bias=mbg_sb[:, j:j + 1])
                wv = wmp.tile([128, D], BF16, tag="wm", name=f"wv{j}")
                nc.sync.dma_start(wv, wm[(16 + j) * 128:(17 + j) * 128, :])
                accv = psA.tile([128, B], F32, tag="acc")
                for k in range(IT):
                    nc.tensor.matmul(accv, wv[:, k * 128:(k + 1) * 128],
                                     xb[k], start=(k == 0),
                                     stop=(k == IT - 1))
                ht = hp.tile([128, B], F32, tag="h", name=f"h0_{j}")
                nc.vector.scalar_tensor_tensor(ht, accv, mbv_sb[:, j:j + 1],
                                               sig, OP.add, OP.mult)
                h_cur.append(ht)
                # A(0,q0) woven in at HALF-unit granularity: a full unit's
                # 34 DVE ops between consecutive h0 evac STTs starves the
                # PSUM pool and stalls the PE; half units keep the DVE lag
                # under the psA slack. na=0: no ln/exp while the sigmoid
                # table-set is live. Quarter 0 only: more would exhaust
                # bas slots (shared with x tiles).
                if 2 <= j < 2 + 2 * KQ:
                    u = (j - 2) // 2
                    if (j - 2) % 2 == 0:
                        emit_A_z(0, u, h_cur[u], 0)
                    else:
                        emit_A_casc(0, u)
            # zero tile written only after the last mult evac: gates the
            # silu batch out of the sigmoid-set window
            z00 = qp.tile([128, 1], F32, tag="zb", bufs=2)
            nc.vector.tensor_scalar(z00, h_cur[IT - 1][:, 0:1], 0.0, None,
                                    OP.mult)
            for j in range(IT):
                emit_silu(0, j, h_cur[j], bias=z00)

            # ---- KAN layers: 4-quarter k-split sweeps ----
            h3 = []
            for l in range(3):
                ot = WIDTHS[l] // 128
                hacc = [None] * ot
                for q in range(NQ):
                    zb = None
                    if l + 1 < 3 and q == NQ - 1:
                        # gate for next layer's silu batch: ready only after
                        # this layer's last bases tile (i.e. after all ln/exp
                        # ACT work for layer l is done)
                        zb = qp.tile([128, 1], F32, tag="zb", bufs=2,
                                     name=f"zb{l}")
                        nc.vector.tensor_scalar(
                            zb, bas_t[(l, IT - 1, COEFF - 1)][:, 0:1], 0.0,
                            None, OP.mult)
                    for o in range(ot):
                        strip = wsp.tile([128, SW], BF16, tag="ws",
                                         name=f"ws{l}_{q}_{o}")
                        row = (o * NQ + q) * 128
                        nc.sync.dma_start(strip, ws_d[l][row:row + 128, :])
                        acc = psA.tile([128, B], F32, tag="acc")
                        idx = 0
                        last = KQ * (1 + COEFF) - 1
                        for kk in range(KQ):
                            i = q * KQ + kk
                            nc.tensor.matmul(
                                acc, strip[:, kk * 128:(kk + 1) * 128],
                                silu_t[(l, i)], start=(idx == 0),
                                stop=(idx == last))
                            idx += 1
                            for c in range(COEFF):
                                o0 = 512 + (kk * COEFF + c) * 128
                                nc.tensor.matmul(
                                    acc, strip[:, o0:o0 + 128],
                                    bas_t[(l, i, c)], start=False,
                                    stop=(idx == last))
                                idx += 1
                        if q == 0:
                            hacc[o] = hp.tile([128, B], F32, tag="h",
                                              name=f"hacc{l}_{o}")
                            nc.scalar.copy(hacc[o], acc)
                        elif q < NQ - 1 or l < 2:
                            nc.vector.tensor_tensor(hacc[o], acc, hacc[o],
                                                    OP.add)
                        else:
                            h3t = silup.tile([128, B], BF16, tag="silu",
                                             name=f"h3_{o}")
                            nc.vector.tensor_tensor(h3t, acc, hacc[o],
                                                    OP.add)
                            h3.append(h3t)
                            # interleave head matmuls into the final sweep
                            if o == 0:
                                acch = psH.tile([128, B], F32, tag="acch")
                            nc.tensor.matmul(acch[0:2, :],
                                             wh_sb[:, 2 * o:2 * o + 2],
                                             h3t, start=(o == 0),
                                             stop=(o == ot - 1))
                        # weave bases one quarter ahead: during B(l,q) build
                        # bases for quarter q+1 (slots freed by sweep q-1).
                        # At q3, batch next layer's silus (table-set
                        # grouping) and its quarter-0 bases.
                        if l + 1 < 3 and q == NQ - 1:
                            emit_silu(l + 1, o, hacc[o], bias=zb)
                            if o < KQ:
                                # na=7 balances DVE vs ACT in the boundary
                                # window (costs a couple of set switches)
                                emit_A(l + 1, o, hacc[o], 7)
                        if q < NQ - 1 and o < KQ:
                            na = 0 if (l == 0 and q == 0) else \
                                8 if l == 2 else 5
                            emit_A(l, KQ * (q + 1) + o,
                                   h_cur[KQ * (q + 1) + o], na)
                h_cur = hacc

            # ---- heads (matmuls interleaved into the last sweep) ----
            res = consts.tile([2, B], F32, tag="res")
            nc.vector.tensor_scalar(res, acch[0:2, :], hb_sb[:, 0:1], None,
                                    OP.add)
            nc.sync.dma_start(out[:], res)

    return _patch_json(nc)


def _prep(inputs):
    """Host-side weight prep: fold scaler+lam, transpose, tile, bf16-cast."""
    f32 = np.float32
    bf16 = ml_dtypes.bfloat16
    feed = {}

    mw = np.asarray(inputs["mult_w"], f32).copy()  # [4096, 2048]
    mw[D:] *= LAM
    feed["wm"] = np.ascontiguousarray(
        mw.reshape(32, 128, IT, 128).transpose(0, 3, 2, 1)
        .reshape(32 * 128, D)).astype(bf16)
    mb = np.asarray(inputs["mult_b"], f32)
    feed["mbg"] = np.ascontiguousarray(mb[:D].reshape(16, 128).T).astype(f32)
    feed["mbv"] = np.ascontiguousarray(
        (LAM * mb[D:]).reshape(16, 128).T).astype(f32)

    for l, fo in enumerate(WIDTHS):
        sc_out = LAM if l < 2 else 1.0
        bw = np.asarray(inputs[f"base_w{l}"], f32) * sc_out
        sw = (np.asarray(inputs[f"spline_w{l}"], f32)
              * np.asarray(inputs[f"scaler{l}"], f32)[..., None] * sc_out)
        ot = fo // 128
        bwt = bw.reshape(ot, 128, IT, 128).transpose(0, 3, 2, 1)
        swt = sw.reshape(ot, 128, IT, 128, COEFF).transpose(0, 3, 2, 4, 1)
        arr = np.empty((ot, NQ, 128, SW), f32)
        arr[:, :, :, :512] = (bwt.reshape(ot, 128, NQ, KQ, 128)
                              .transpose(0, 2, 1, 3, 4)
                              .reshape(ot, NQ, 128, KQ * 128))
        arr[:, :, :, 512:] = (swt.reshape(ot, 128, NQ, KQ, COEFF, 128)
                              .transpose(0, 2, 1, 3, 4, 5)
                              .reshape(ot, NQ, 128, KQ * COEFF * 128))
        feed[f"ws{l}"] = np.ascontiguousarray(
            arr.reshape(ot * NQ * 128, SW)).astype(bf16)

    whh = np.stack([np.asarray(inputs["reg_w"], f32)[0],
                    np.asarray(inputs["aux_w"], f32)[0]], axis=1)  # [1024,2]
    feed["wh"] = np.ascontiguousarray(
        whh.reshape(8, 128, 2).transpose(1, 0, 2).reshape(128, 16)
    ).astype(bf16)
    feed["hb"] = np.array([[np.asarray(inputs["reg_b"], f32)[0]],
                           [np.asarray(inputs["aux_b"], f32)[0]]], f32)
    return feed


_NC = None


def kernel(**inputs):
    global _NC
    from concourse.bass_utils import run_bass_kernel_spmd

    if _NC is None:
        _NC = build()
    shared = _prep(inputs)
    x_full = np.asarray(inputs["x"], np.float32)
    per_core = []
    for c in range(N_CORES):
        m = dict(shared)
        m["xT"] = np.ascontiguousarray(
            x_full[c * B:(c + 1) * B].T).astype(ml_dtypes.bfloat16)
        per_core.append(m)
    res = run_bass_kernel_spmd(_NC, per_core, core_ids=list(range(N_CORES)))
    reg = np.concatenate([res.results[c]["out"][0] for c in range(N_CORES)])
    aux = np.concatenate([res.results[c]["out"][1] for c in range(N_CORES)])
    kernel.last_results = res
    return reg, aux



# revision 2
# speedup vs baseline: 1.0380x; 1.0380x over previous
"""BRD4KANModel Trainium2 kernel — fp8 DoubleRow spline edition.

Data-parallel over batch across 8 NeuronCores (512 rows each, weights
replicated). Three levers over the bf16 baseline:

1. Spline matmuls run in fp8(e4m3) DoubleRow perf mode (0.5 PE cycles/row):
   each instruction contracts a PAIR of 128-deep k-groups. Layer 0 pairs
   adjacent coefficients (b_c, b_{c+1}); layers 1-2 pair (q4, d4) where
   q4 = fp8(b4) and d4 = fp8(b4 - q4) — a value+residual split that
   recovers ~fp16 basis precision at fp8 speed (both groups share the same
   weight column). Spline weights are LS-quantized to fp8: greedy
   coordinate descent against the basis Gram matrix (errors that typical
   basis vectors can "see" are minimized, ~1.6x better than RNE).

2. B-spline bases via the symmetric closed form instead of the truncated
   power cascade: with u = |s-2| in knot units, b = [(2-u)+^3 -4(1-u)+^3]/6.
   Both truncated cubes are O(1) (no cancellation), so the whole chain
   runs in fp16 where DVE has 2x/4x perf modes: u on ACT (Abs, every table
   set), v/w clamps on DVE tensor_scalar (4x), squares on ACT (the x4 of
   the second cube folds into Square's input scale=2), cubes and the final
   subtract on DVE tensor_tensor (2x). scalar_tensor_tensor is avoided in
   the hot path (it has no DVE fast mode).

3. fp8 weights need a 2^j range scale; base_w is scaled by the same 2^j on
   the host so base+spline accumulate in one PSUM bank, and the single
   evacuation op applies 2^-j (folded into the existing ACT Copy / DVE STT).
   h tiles are fp16 (halves SBUF) — h only feeds ACT ops (dtype-free).

Schedule skeleton (emission order) follows the proven baseline: 4-quarter
k-split sweeps, bases for quarter q+1 woven into quarter q's sweeps, next
layer's silus+quarter-0 bases at q=3. ACT table sets: Abs/Square/Copy live
in every set, so only the one sigmoid->silu switch after the mult layer
remains (l0 silus gated behind a zero tile to keep them out of the sigmoid
window).

This walrus build accepts only ONE semaphore wait per instruction;
_split_waits() post-processes the BIR JSON as in the baseline.
"""

import json

import numpy as np
import ml_dtypes

import concourse.bass as bass
import concourse.mybir as mybir
import concourse.tile as tile

F32 = mybir.dt.float32
BF16 = mybir.dt.bfloat16
F16 = mybir.dt.float16
F8 = mybir.dt.float8e4
AF = mybir.ActivationFunctionType
OP = mybir.AluOpType
DR = mybir.MatmulPerfMode.DoubleRow

N_CORES = 8
BATCH = 4096
B = BATCH // N_CORES  # 512 per core
D = 2048
WIDTHS = [2048, 2048, 1024]
COEFF = 6
IT = 16          # 2048/128 input tiles per layer
NQ = 4           # k-quarters
KQ = IT // NQ    # i-tiles per quarter

HSTEP = 2.0 / 3.0
KNOT = [m * HSTEP - 1.0 - 3 * HSTEP for m in range(10)]  # 10 knots, -3..3
CS = (4.0 / 6.0) ** (1.0 / 3.0)   # folds the /6 and x4 into the cubes
SCL_IN = 1.5 * CS                 # h -> u input scale (1/HSTEP * CS)
A2 = -2.0 * CS
A1 = -1.0 * CS

MODES = ["plain", "resid", "resid"]
NPAIRS = {"plain": 3, "resid": 6}
WQCOLS = KQ * 6 * 2 * 128         # resid strip width (l0 uses half)


def _split_waits(bir_bytes: bytes, keep: int = 1) -> bytes:
    d = json.loads(bir_bytes)
    for f in d["functions"]:
        for bb in f["blocks"]:
            new_insts = []
            for inst in bb["instructions"]:
                si = inst.get("sync_info")
                waits = (si or {}).get("on_wait") or []
                if len(waits) > keep:
                    extra = waits[:-keep]
                    inst["sync_info"]["on_wait"] = waits[-keep:]
                    for ci in range(0, len(extra), keep):
                        new_insts.append({
                            "name": f"{inst['name']}-w{ci}",
                            "opcode": "NoOp",
                            "engine": inst["engine"],
                            "ins": [],
                            "outs": [],
                            "debug": inst.get("debug"),
                            "sync_info": {"on_update": [],
                                          "on_wait": extra[ci:ci + keep]},
                        })
                new_insts.append(inst)
            bb["instructions"] = new_insts
    return json.dumps(d).encode()


def _patch_json(nc):
    orig = nc.to_json_bytes

    def patched():
        return _split_waits(orig())

    nc.to_json_bytes = patched
    return nc


def build(js):
    sc = [float(2.0 ** -j) for j in js]   # per-layer evac scales
    nc = bass.Bass()
    xT = nc.dram_tensor("xT", [D, B], BF16, kind="ExternalInput")
    wm = nc.dram_tensor("wm", [32 * 128, D], BF16, kind="ExternalInput")
    mbg = nc.dram_tensor("mbg", [128, 16], F32, kind="ExternalInput")
    mbv = nc.dram_tensor("mbv", [128, 16], F32, kind="ExternalInput")
    wb_d, wq_d = [], []
    for l, fo in enumerate(WIDTHS):
        ot = fo // 128
        ncols = KQ * NPAIRS[MODES[l]] * 2 * 128
        wb_d.append(nc.dram_tensor(f"wb{l}", [ot * NQ * 128, KQ * 128], BF16,
                                   kind="ExternalInput"))
        wq_d.append(nc.dram_tensor(f"wq{l}", [ot * NQ * 128, ncols], F8,
                                   kind="ExternalInput"))
    wh = nc.dram_tensor("wh", [128, 16], BF16, kind="ExternalInput")
    hb = nc.dram_tensor("hb", [2, 1], F32, kind="ExternalInput")
    out = nc.dram_tensor("out", [2, B], F32, kind="ExternalOutput")

    with tile.TileContext(nc) as tc:
        with tc.tile_pool(name="consts", bufs=1) as consts, \
             tc.tile_pool(name="wmp", bufs=2) as wmp, \
             tc.tile_pool(name="wbp", bufs=3) as wbp, \
             tc.tile_pool(name="wqp", bufs=3) as wqp, \
             tc.tile_pool(name="xsp", bufs=24) as xsp, \
             tc.tile_pool(name="h0p", bufs=16) as h0p, \
             tc.tile_pool(name="hap", bufs=25) as hap, \
             tc.tile_pool(name="bp1", bufs=27) as bp1, \
             tc.tile_pool(name="bp6", bufs=9) as bp6, \
             tc.tile_pool(name="tp", bufs=10) as tp, \
             tc.tile_pool(name="qp", bufs=6) as qp, \
             tc.tile_pool(name="psA", bufs=7, space="PSUM") as psA, \
             tc.tile_pool(name="psH", bufs=1, space="PSUM") as psH:

            # ---- constants ----
            dcm = consts.tile([128, COEFF], F32, tag="dcm")
            for c in range(COEFF):
                nc.vector.memset(dcm[:, c:c + 1],
                                 float(-KNOT[c + 2] * SCL_IN))
            mbg_sb = consts.tile([128, 16], F32, tag="mbg")
            nc.scalar.dma_start(mbg_sb, mbg[:])
            mbv_sb = consts.tile([128, 16], F32, tag="mbv")
            nc.scalar.dma_start(mbv_sb, mbv[:])
            wh_sb = consts.tile([128, 16], BF16, tag="wh")
            nc.scalar.dma_start(wh_sb, wh[:])
            hb_sb = consts.tile([2, 1], F32, tag="hb")
            nc.scalar.dma_start(hb_sb, hb[:])

            # ---- x^T tiles (host pre-transposed; slots later reused by
            # silu/h3 tiles via the shared "xs" tag) ----
            xb = []
            for i in range(IT):
                t = xsp.tile([128, B], BF16, tag="xs", name=f"x{i}")
                nc.scalar.dma_start(t, xT[i * 128:(i + 1) * 128, :])
                xb.append(t)

            # warmup: keep the PE busy through the DMA startup
            wz = consts.tile([128, 128], BF16, tag="wz")
            nc.vector.memset(wz, 0.0)
            accw = psH.tile([128, B], F32, tag="acch", name="warm")
            for k in range(60):
                nc.tensor.matmul(accw[:, 0:128], wz, wz, start=(k == 0),
                                 stop=(k == 59))

            silu_t = {}
            pair_t = {}   # (l, i, pr) -> [128, 2, B] fp8 (plain)
            pair6_t = {}  # (l, i) -> [128, 6, 2, B] fp8 (resid)

            def emit_silu(l, i, h_t, bias=0.0):
                st = xsp.tile([128, B], BF16, tag="xs", name=f"silu{l}_{i}")
                nc.scalar.activation(st, h_t, AF.Silu, bias=bias)
                silu_t[(l, i)] = st

            def emit_pair(l, i, h_t, pr):
                """Bases for coefficient pair (2pr, 2pr+1) of unit (l,i)."""
                mode = MODES[l]
                u2 = tp.tile([128, 2, B], F16, tag="tt", name=f"u{i}_{pr}")
                for g in range(2):
                    c = 2 * pr + g
                    nc.scalar.activation(u2[:, g, :], h_t, AF.Abs,
                                         scale=SCL_IN, bias=dcm[:, c:c + 1])
                vp = tp.tile([128, 2, B], F16, tag="tt", name=f"v{i}_{pr}")
                nc.vector.tensor_scalar(vp, u2, A2, 0.0, OP.add, OP.min)
                wp = tp.tile([128, 2, B], F16, tag="tt", name=f"w{i}_{pr}")
                nc.vector.tensor_scalar(wp, u2, A1, 0.0, OP.add, OP.min)
                sv = tp.tile([128, 2, B], F16, tag="tt", name=f"sv{i}_{pr}")
                nc.scalar.activation(sv, vp, AF.Square)
                sw = tp.tile([128, 2, B], F16, tag="tt", name=f"sw{i}_{pr}")
                nc.scalar.activation(sw, wp, AF.Square, scale=2.0)
                v3 = tp.tile([128, 2, B], F16, tag="tt", name=f"v3{i}_{pr}")
                nc.vector.tensor_tensor(v3, vp, sv, OP.mult)
                w3 = tp.tile([128, 2, B], F16, tag="tt", name=f"w3{i}_{pr}")
                nc.vector.tensor_tensor(w3, wp, sw, OP.mult)
                if mode == "plain":
                    bp = bp1.tile([128, 2, B], F8, tag="b1",
                                  name=f"bp{l}_{i}_{pr}")
                    nc.vector.tensor_tensor(bp, w3, v3, OP.subtract)
                    pair_t[(l, i, pr)] = bp
                else:
                    b4 = tp.tile([128, 2, B], F16, tag="tt",
                                 name=f"b4{i}_{pr}")
                    nc.vector.tensor_tensor(b4, w3, v3, OP.subtract)
                    pt = pair6_t[(l, i)]
                    qap = pt[:, 2 * pr:2 * pr + 2, 0, :]
                    nc.scalar.activation(qap, b4, AF.Copy)
                    nc.vector.tensor_tensor(pt[:, 2 * pr:2 * pr + 2, 1, :],
                                            b4, qap, OP.subtract)

            def emit_unit(l, i, h_t, prs=(0, 1, 2)):
                if MODES[l] == "resid" and (l, i) not in pair6_t:
                    pair6_t[(l, i)] = bp6.tile([128, COEFF, 2, B], F8,
                                               tag="b6", name=f"p6_{l}_{i}")
                for pr in prs:
                    emit_pair(l, i, h_t, pr)

            # ---- multiplicative layer ----
            h_cur = []
            for j in range(IT):
                wg = wmp.tile([128, D], BF16, tag="wm", name=f"wg{j}")
                nc.sync.dma_start(wg, wm[j * 128:(j + 1) * 128, :])
                accg = psA.tile([128, B], F32, tag="acc")
                for k in range(IT):
                    nc.tensor.matmul(accg, wg[:, k * 128:(k + 1) * 128],
                                     xb[k], start=(k == 0),
                                     stop=(k == IT - 1))
                sig = qp.tile([128, B], F16, tag="q", name=f"sig{j}")
                nc.scalar.activation(sig, accg, AF.Sigmoid,
                                     bias=mbg_sb[:, j:j + 1])
                wv = wmp.tile([128, D], BF16, tag="wm", name=f"wv{j}")
                nc.sync.dma_start(wv, wm[(16 + j) * 128:(17 + j) * 128, :])
                accv = psA.tile([128, B], F32, tag="acc")
                for k in range(IT):
                    nc.tensor.matmul(accv, wv[:, k * 128:(k + 1) * 128],
                                     xb[k], start=(k == 0),
                                     stop=(k == IT - 1))
                ht = h0p.tile([128, B], F16, tag="h0", name=f"h0_{j}")
                nc.vector.scalar_tensor_tensor(ht, accv, mbv_sb[:, j:j + 1],
                                               sig, OP.add, OP.mult)
                h_cur.append(ht)
                # weave l0 quarter-0 bases at half-unit granularity so the
                # DVE lag between consecutive evac STTs stays under the psA
                # slack
                if 8 <= j < 16:
                    u = (j - 8) // 2
                    if (j - 8) % 2 == 0:
                        emit_unit(0, u, h_cur[u], prs=(0, 1))
                    else:
                        emit_unit(0, u, h_cur[u], prs=(2,))
            # zero tile written only after the last mult evac: keeps the silu
            # batch (one table switch) out of the sigmoid-set window
            z00 = qp.tile([128, 1], F32, tag="zb", bufs=2)
            nc.vector.tensor_scalar(z00, h_cur[IT - 1][:, 0:1], 0.0, None,
                                    OP.mult)
            for j in range(IT):
                emit_silu(0, j, h_cur[j], bias=z00)

            # ---- KAN layers: 4-quarter k-split sweeps ----
            h3 = []
            for l in range(3):
                ot = WIDTHS[l] // 128
                mode = MODES[l]
                npair = NPAIRS[mode]
                scl = sc[l]
                hacc = [None] * ot
                for q in range(NQ):
                    for o in range(ot):
                        row = (o * NQ + q) * 128
                        wbs = wbp.tile([128, KQ * 128], BF16, tag="wb",
                                       name=f"wb{l}_{q}_{o}")
                        nc.sync.dma_start(wbs, wb_d[l][row:row + 128, :])
                        ncols = KQ * npair * 2 * 128
                        wqs = wqp.tile([128, WQCOLS], F8, tag="wq",
                                       name=f"wq{l}_{q}_{o}")
                        nc.sync.dma_start(wqs[:, :ncols],
                                          wq_d[l][row:row + 128, :])
                        acc = psA.tile([128, B], F32, tag="acc")
                        idx = 0
                        last = KQ * (1 + npair) - 1
                        for kk in range(KQ):
                            i = q * KQ + kk
                            nc.tensor.matmul(
                                acc, wbs[:, kk * 128:(kk + 1) * 128],
                                silu_t[(l, i)], start=(idx == 0),
                                stop=(idx == last))
                            idx += 1
                            for pp in range(npair):
                                o0 = (kk * npair + pp) * 256
                                lhs = wqs[:, o0:o0 + 256].rearrange(
                                    "p (two m) -> p two m", two=2)
                                if mode == "plain":
                                    rhs = pair_t[(l, i, pp)][:, :, :]
                                else:
                                    rhs = pair6_t[(l, i)][:, pp, :, :]
                                nc.tensor.matmul(acc, lhs, rhs, start=False,
                                                 stop=(idx == last),
                                                 perf_mode=DR)
                                idx += 1
                        if q == 0:
                            hacc[o] = hap.tile([128, B], F16, tag="ha",
                                               name=f"ha{l}_{o}")
                            nc.scalar.activation(hacc[o], acc, AF.Copy,
                                                 scale=scl)
                        elif q < NQ - 1 or l < 2:
                            nc.vector.scalar_tensor_tensor(
                                hacc[o], acc, scl, hacc[o], OP.mult, OP.add)
                        else:
                            h3t = xsp.tile([128, B], BF16, tag="xs",
                                           name=f"h3_{o}")
                            nc.vector.scalar_tensor_tensor(
                                h3t, acc, scl, hacc[o], OP.mult, OP.add)
                            h3.append(h3t)
                            # interleave head matmuls into the final sweep
                            if o == 0:
                                acch = psH.tile([128, B], F32, tag="acch")
                            nc.tensor.matmul(acch[0:2, :],
                                             wh_sb[:, 2 * o:2 * o + 2],
                                             h3t, start=(o == 0),
                                             stop=(o == ot - 1))
                        # weave bases one quarter ahead; at q3 batch the next
                        # layer's silus and its quarter-0 bases
                        if q < NQ - 1 and o < KQ:
                            emit_unit(l, KQ * (q + 1) + o,
                                      h_cur[KQ * (q + 1) + o])
                        if q == NQ - 1 and l < 2:
                            emit_silu(l + 1, o, hacc[o])
                            if o < KQ:
                                emit_unit(l + 1, o, hacc[o])
                h_cur = hacc

            # ---- heads ----
            res = consts.tile([2, B], F32, tag="res")
            nc.vector.tensor_scalar(res, acch[0:2, :], hb_sb[:, 0:1], None,
                                    OP.add)
            nc.sync.dma_start(out[:], res)

    return _patch_json(nc)


# ---------------- host-side prep ----------------

_f32 = np.float32
_bf16 = ml_dtypes.bfloat16
_f8 = ml_dtypes.float8_e4m3


def _bases_np(h):
    """Closed-form b4 = 4*bases, numpy float32 (for the Gram matrix)."""
    out = np.empty(h.shape + (COEFF,), _f32)
    for c in range(COEFF):
        u = np.abs(h * _f32(SCL_IN) + _f32(-KNOT[c + 2] * SCL_IN))
        v = np.minimum(u + _f32(A2), 0.0)
        w = np.minimum(u + _f32(A1), 0.0)
        out[..., c] = (2.0 * w) ** 2 * w - v ** 3
    return out


def _gram():
    hs = (np.random.default_rng(7).standard_normal(100000) * 1.2).astype(_f32)
    b = _bases_np(hs)
    return (b.T @ b / len(b)).astype(_f32)


def _ls_q8(w_scaled, M):
    """Greedy fp8 quantization of (..., 6) weight vectors minimizing the
    quadratic form with basis Gram M. Returns fp8-representable float32."""
    sh = w_scaled.shape
    W = w_scaled.reshape(-1, COEFF).astype(_f32)
    Q = W.astype(_f8).astype(_f32)
    big_up = np.array(1000.0, _f8)
    big_dn = np.array(-1000.0, _f8)
    for _ in range(2):
        for c in range(COEFF):
            g = (Q - W) @ M[:, c]
            w8 = Q[:, c].astype(_f8)
            stepped = np.where(g > 0, np.nextafter(w8, big_dn),
                               np.nextafter(w8, big_up)).astype(_f32)
            dq = stepped - Q[:, c]
            dcost = 2 * dq * g + dq * dq * M[c, c]
            take = dcost < 0
            Q[:, c] = np.where(take, stepped, Q[:, c])
    return Q.reshape(sh)


def _prep(inputs):
    feed = {}
    mw = np.asarray(inputs["mult_w"], _f32)
    feed["wm"] = np.ascontiguousarray(
        mw.reshape(32, 128, IT, 128).transpose(0, 3, 2, 1)
        .reshape(32 * 128, D)).astype(_bf16)
    mb = np.asarray(inputs["mult_b"], _f32)
    feed["mbg"] = np.ascontiguousarray(mb[:D].reshape(16, 128).T).astype(_f32)
    feed["mbv"] = np.ascontiguousarray(mb[D:].reshape(16, 128).T).astype(_f32)

    M = _gram()
    js = []
    for l, fo in enumerate(WIDTHS):
        fi = ([D] + WIDTHS)[l]
        ot, itl = fo // 128, fi // 128
        sw = (np.asarray(inputs[f"spline_w{l}"], _f32)
              * np.asarray(inputs[f"scaler{l}"], _f32)[..., None]) / 4.0
        j = int(np.floor(np.log2(224.0 / np.abs(sw).max())))
        js.append(j)
        s = _f32(2.0 ** j)

        bw = np.asarray(inputs[f"base_w{l}"], _f32) * s
        bwt = bw.reshape(ot, 128, itl, 128)          # [o, oc, it, p]
        arr = (bwt.reshape(ot, 128, NQ, KQ, 128)
               .transpose(0, 2, 4, 3, 1)             # [o, q, p, kk, oc]
               .reshape(ot * NQ * 128, KQ * 128))
        feed[f"wb{l}"] = np.ascontiguousarray(arr).astype(_bf16)

        swq = _ls_q8(sw * s, M)                      # [fo, fi, 6] fp8 values
        if MODES[l] == "plain":
            cmap = np.array([[0, 1], [2, 3], [4, 5]])
        else:
            cmap = np.array([[0, 0], [1, 1], [2, 2], [3, 3], [4, 4], [5, 5]])
        npair = cmap.shape[0]
        swt = swq.reshape(ot, 128, itl, 128, COEFF)  # [o, oc, it, p, c]
        sel = swt[:, :, :, :, cmap]                  # [o, oc, it, p, pp, g]
        arr = (sel.reshape(ot, 128, NQ, KQ, 128, npair, 2)
               .transpose(0, 2, 4, 3, 5, 6, 1)       # [o,q,p,kk,pp,g,oc]
               .reshape(ot * NQ * 128, KQ * npair * 2 * 128))
        feed[f"wq{l}"] = np.ascontiguousarray(arr).astype(_f8)

    whh = np.stack([np.asarray(inputs["reg_w"], _f32)[0],
                    np.asarray(inputs["aux_w"], _f32)[0]], axis=1)  # [1024,2]
    feed["wh"] = np.ascontiguousarray(
        whh.reshape(8, 128, 2).transpose(1, 0, 2).reshape(128, 16)
    ).astype(_bf16)
    feed["hb"] = np.array([[np.asarray(inputs["reg_b"], _f32)[0]],
                           [np.asarray(inputs["aux_b"], _f32)[0]]], _f32)
    return feed, tuple(js)


_NC = {}


def kernel(**inputs):
    from concourse.bass_utils import run_bass_kernel_spmd

    shared, js = _prep(inputs)
    if js not in _NC:
        _NC[js] = build(js)
    x_full = np.asarray(inputs["x"], np.float32)
    per_core = []
    for c in range(N_CORES):
        m = dict(shared)
        m["xT"] = np.ascontiguousarray(
            x_full[c * B:(c + 1) * B].T).astype(_bf16)
        per_core.append(m)
    res = run_bass_kernel_spmd(_NC[js], per_core, core_ids=list(range(N_CORES)))
    reg = np.concatenate([res.results[c]["out"][0] for c in range(N_CORES)])
    aux = np.concatenate([res.results[c]["out"][1] for c in range(N_CORES)])
    kernel.last_results = res
    return reg, aux


# revision 4
# speedup vs baseline: 1.2890x; 1.2418x over previous
"""BRD4KANModel Trainium2 kernel — fp8 DoubleRow spline edition.

Data-parallel over batch across 8 NeuronCores (512 rows each, weights
replicated). Three levers over the bf16 baseline:

1. Spline matmuls for layers 0-1 run in fp8(e4m3) DoubleRow perf mode: one
   instruction contracts a PAIR of 128-deep k-groups (adjacent coefficients
   b_c, b_{c+1}) in ~the time of one bf16 K=128 matmul — 2x spline flops.
   (On real HW a DR instruction costs ~239ns vs 225ns bf16 — the fp8 ifmap
   streams 2 rows/cycle — so only plain pairing wins; a value+residual
   pairing doubles k-groups and gains nothing.) Layer 2's spline errors hit
   the output un-attenuated, so it stays bf16 (its 6 coefficient columns
   ride in the bf16 strip). Spline weights are LS-quantized to fp8: greedy
   coordinate descent against the basis Gram matrix, ~1.6x better than RNE.

2. B-spline bases via the symmetric closed form instead of the truncated
   power cascade: with u = |s-2| in knot units, b = [(2-u)+^3 -4(1-u)+^3]/6.
   Both truncated cubes are O(1) (no cancellation), so the whole chain runs
   in fp16 where DVE has 2x/4x perf modes: u on ACT (Abs lives in every
   table set), v/w clamps on DVE tensor_scalar (4x), squares on ACT (the x4
   of the second cube folds into Square's input scale=2), cubes and the
   final subtract on DVE tensor_tensor (2x). scalar_tensor_tensor is
   avoided in the hot path (it has no DVE fast mode). Bases are emitted as
   4*b (pairs with the /4-folded weights) straight to fp8/bf16 pair tiles.

3. fp8 weights need a 2^j range scale; base_w (and the bf16 spline columns)
   are scaled by the same 2^j on the host so everything accumulates in one
   PSUM bank, and the single evacuation op applies 2^-j (folded into the
   existing ACT Copy / DVE STT). h tiles are fp16 (halves SBUF) — h only
   feeds ACT ops (dtype-free) and fp16 quarter-accumulation error is ~8e-4.

Schedule skeleton follows the proven baseline: 4-quarter k-split sweeps,
bases for quarter q+1 woven into quarter q's sweeps, next layer's silus +
quarter-0 bases at q=3. ACT table sets: Abs/Square/Copy live in every set,
so only the one sigmoid->silu switch after the mult layer remains (l0 silus
gated behind a zero tile to keep them out of the sigmoid window).

This walrus build accepts only ONE semaphore wait per instruction;
_split_waits() post-processes the BIR JSON as in the baseline.
"""

import json

import numpy as np
import ml_dtypes

import concourse.bass as bass
import concourse.mybir as mybir
import concourse.tile as tile

F32 = mybir.dt.float32
BF16 = mybir.dt.bfloat16
F16 = mybir.dt.float16
F8 = mybir.dt.float8e4
AF = mybir.ActivationFunctionType
OP = mybir.AluOpType
DR = mybir.MatmulPerfMode.DoubleRow

N_CORES = 8
BATCH = 4096
B = BATCH // N_CORES  # 512 per core
D = 2048
WIDTHS = [2048, 2048, 1024]
COEFF = 6
IT = 16          # 2048/128 input tiles per layer
NQ = 4           # k-quarters
KQ = IT // NQ    # i-tiles per quarter

HSTEP = 2.0 / 3.0
KNOT = [m * HSTEP - 1.0 - 3 * HSTEP for m in range(10)]  # 10 knots, -3..3
CS = (4.0 / 6.0) ** (1.0 / 3.0)   # folds the /6 and x4 into the cubes
SCL_IN = 1.5 * CS                 # h -> u input scale (1/HSTEP * CS)
A2 = -2.0 * CS
A1 = -1.0 * CS

NPLAIN = [6, 6, 0]                # fp8-DR coeffs per layer (rest bf16)
NDRP = [n // 2 for n in NPLAIN]   # DR pair-instructions per i-tile
NBF = [COEFF - n for n in NPLAIN]
WBCOLS = [KQ * (1 + nb) * 128 for nb in NBF]
WQCOLS = [KQ * nd * 2 * 128 for nd in NDRP]


def _split_waits(bir_bytes: bytes, keep: int = 1) -> bytes:
    d = json.loads(bir_bytes)
    for f in d["functions"]:
        for bb in f["blocks"]:
            new_insts = []
            for inst in bb["instructions"]:
                si = inst.get("sync_info")
                waits = (si or {}).get("on_wait") or []
                if len(waits) > keep:
                    extra = waits[:-keep]
                    inst["sync_info"]["on_wait"] = waits[-keep:]
                    for ci in range(0, len(extra), keep):
                        new_insts.append({
                            "name": f"{inst['name']}-w{ci}",
                            "opcode": "NoOp",
                            "engine": inst["engine"],
                            "ins": [],
                            "outs": [],
                            "debug": inst.get("debug"),
                            "sync_info": {"on_update": [],
                                          "on_wait": extra[ci:ci + keep]},
                        })
                new_insts.append(inst)
            bb["instructions"] = new_insts
    return json.dumps(d).encode()


def _patch_json(nc):
    orig = nc.to_json_bytes

    def patched():
        return _split_waits(orig())

    nc.to_json_bytes = patched
    return nc


def build(js):
    sc = [float(2.0 ** -j) for j in js]   # per-layer evac scales
    nc = bass.Bass()
    xT = nc.dram_tensor("xT", [D, B], BF16, kind="ExternalInput")
    wm = nc.dram_tensor("wm", [32 * 128, D], BF16, kind="ExternalInput")
    mbg = nc.dram_tensor("mbg", [128, 16], F32, kind="ExternalInput")
    mbv = nc.dram_tensor("mbv", [128, 16], F32, kind="ExternalInput")
    wb_d, wq_d = [], []
    for l, fo in enumerate(WIDTHS):
        ot = fo // 128
        wb_d.append(nc.dram_tensor(f"wb{l}", [ot * NQ * 128, WBCOLS[l]],
                                   BF16, kind="ExternalInput"))
        wq_d.append(nc.dram_tensor(f"wq{l}", [ot * NQ * 128, WQCOLS[l]], F8,
                                   kind="ExternalInput")
                    if NDRP[l] else None)
    wh = nc.dram_tensor("wh", [128, 16], BF16, kind="ExternalInput")
    hb = nc.dram_tensor("hb", [2, 1], F32, kind="ExternalInput")
    out = nc.dram_tensor("out", [2, B], F32, kind="ExternalOutput")

    with tile.TileContext(nc) as tc:
        with tc.tile_pool(name="consts", bufs=1) as consts, \
             tc.tile_pool(name="wmp", bufs=2) as wmp, \
             tc.tile_pool(name="wbp", bufs=3) as wbp, \
             tc.tile_pool(name="wqp", bufs=3) as wqp, \
             tc.tile_pool(name="xsp", bufs=24) as xsp, \
             tc.tile_pool(name="h0p", bufs=16) as h0p, \
             tc.tile_pool(name="hap", bufs=25) as hap, \
             tc.tile_pool(name="bp1", bufs=27) as bp1, \
             tc.tile_pool(name="bp2", bufs=27) as bp2, \
             tc.tile_pool(name="tp", bufs=8) as tp, \
             tc.tile_pool(name="qp", bufs=4) as qp, \
             tc.tile_pool(name="psA", bufs=7, space="PSUM") as psA, \
             tc.tile_pool(name="psH", bufs=1, space="PSUM") as psH:

            # ---- constants ----
            dcm = consts.tile([128, COEFF], F32, tag="dcm")
            for c in range(COEFF):
                nc.vector.memset(dcm[:, c:c + 1],
                                 float(-KNOT[c + 2] * SCL_IN))
            mbg_sb = consts.tile([128, 16], F32, tag="mbg")
            nc.scalar.dma_start(mbg_sb, mbg[:])
            mbv_sb = consts.tile([128, 16], F32, tag="mbv")
            nc.scalar.dma_start(mbv_sb, mbv[:])
            wh_sb = consts.tile([128, 16], BF16, tag="wh")
            nc.scalar.dma_start(wh_sb, wh[:])
            hb_sb = consts.tile([2, 1], F32, tag="hb")
            nc.scalar.dma_start(hb_sb, hb[:])

            # ---- x^T tiles (host pre-transposed; slots later reused by
            # silu/h3 tiles via the shared "xs" tag) ----
            xb = []
            for i in range(IT):
                t = xsp.tile([128, B], BF16, tag="xs", name=f"x{i}")
                nc.scalar.dma_start(t, xT[i * 128:(i + 1) * 128, :])
                xb.append(t)

            # warmup: keep the PE busy through the DMA startup
            wz = consts.tile([128, 128], BF16, tag="wz")
            nc.vector.memset(wz, 0.0)
            accw = psH.tile([128, B], F32, tag="acch", name="warm")
            for k in range(60):
                nc.tensor.matmul(accw[:, 0:128], wz, wz, start=(k == 0),
                                 stop=(k == 59))

            silu_t = {}
            pair_t = {}   # (l, i, pr) -> [128, 2, B] fp8 or bf16 pair tile

            def emit_silu(l, i, h_t, bias=0.0):
                st = xsp.tile([128, B], BF16, tag="xs", name=f"silu{l}_{i}")
                nc.scalar.activation(st, h_t, AF.Silu, bias=bias)
                silu_t[(l, i)] = st

            def emit_pair(l, i, h_t, pr):
                """Bases for coefficient pair (2pr, 2pr+1) of unit (l,i)."""
                u2 = tp.tile([128, 2, B], F16, tag="tt", name=f"u{i}_{pr}")
                for g in range(2):
                    c = 2 * pr + g
                    nc.scalar.activation(u2[:, g, :], h_t, AF.Abs,
                                         scale=SCL_IN, bias=dcm[:, c:c + 1])
                vp = tp.tile([128, 2, B], F16, tag="tt", name=f"v{i}_{pr}")
                nc.vector.tensor_scalar(vp, u2, A2, 0.0, OP.add, OP.min)
                wp = tp.tile([128, 2, B], F16, tag="tt", name=f"w{i}_{pr}")
                nc.vector.tensor_scalar(wp, u2, A1, 0.0, OP.add, OP.min)
                sv = tp.tile([128, 2, B], F16, tag="tt", name=f"sv{i}_{pr}")
                nc.scalar.activation(sv, vp, AF.Square)
                sw = tp.tile([128, 2, B], F16, tag="tt", name=f"sw{i}_{pr}")
                nc.scalar.activation(sw, wp, AF.Square, scale=2.0)
                v3 = tp.tile([128, 2, B], F16, tag="tt", name=f"v3{i}_{pr}")
                nc.vector.tensor_tensor(v3, vp, sv, OP.mult)
                w3 = tp.tile([128, 2, B], F16, tag="tt", name=f"w3{i}_{pr}")
                nc.vector.tensor_tensor(w3, wp, sw, OP.mult)
                if pr < NDRP[l]:
                    bp = bp1.tile([128, 2, B], F8, tag="b1",
                                  name=f"bp{l}_{i}_{pr}")
                else:
                    bp = bp2.tile([128, 2, B], BF16, tag="b2",
                                  name=f"bq{l}_{i}_{pr}")
                nc.vector.tensor_tensor(bp, w3, v3, OP.subtract)
                pair_t[(l, i, pr)] = bp

            def emit_unit(l, i, h_t, prs=(0, 1, 2)):
                for pr in prs:
                    emit_pair(l, i, h_t, pr)

            # ---- multiplicative layer ----
            h_cur = []
            for j in range(IT):
                wg = wmp.tile([128, D], BF16, tag="wm", name=f"wg{j}")
                nc.sync.dma_start(wg, wm[j * 128:(j + 1) * 128, :])
                accg = psA.tile([128, B], F32, tag="acc")
                for k in range(IT):
                    nc.tensor.matmul(accg, wg[:, k * 128:(k + 1) * 128],
                                     xb[k], start=(k == 0),
                                     stop=(k == IT - 1))
                sig = qp.tile([128, B], F16, tag="q", name=f"sig{j}")
                nc.scalar.activation(sig, accg, AF.Sigmoid,
                                     bias=mbg_sb[:, j:j + 1])
                wv = wmp.tile([128, D], BF16, tag="wm", name=f"wv{j}")
                nc.sync.dma_start(wv, wm[(16 + j) * 128:(17 + j) * 128, :])
                accv = psA.tile([128, B], F32, tag="acc")
                for k in range(IT):
                    nc.tensor.matmul(accv, wv[:, k * 128:(k + 1) * 128],
                                     xb[k], start=(k == 0),
                                     stop=(k == IT - 1))
                ht = h0p.tile([128, B], F16, tag="h0", name=f"h0_{j}")
                nc.vector.scalar_tensor_tensor(ht, accv, mbv_sb[:, j:j + 1],
                                               sig, OP.add, OP.mult)
                h_cur.append(ht)
                # weave l0 quarter-0 bases at half-unit granularity so the
                # DVE lag between consecutive evac STTs stays under the psA
                # slack
                if 8 <= j < 16:
                    u = (j - 8) // 2
                    if (j - 8) % 2 == 0:
                        emit_unit(0, u, h_cur[u], prs=(0, 1))
                    else:
                        emit_unit(0, u, h_cur[u], prs=(2,))
            # zero tile written only after the last mult evac: keeps the silu
            # batch (one table switch) out of the sigmoid-set window
            z00 = qp.tile([128, 1], F32, tag="zb", bufs=2)
            nc.vector.tensor_scalar(z00, h_cur[IT - 1][:, 0:1], 0.0, None,
                                    OP.mult)
            for j in range(IT):
                emit_silu(0, j, h_cur[j], bias=z00)

            # ---- KAN layers: 4-quarter k-split sweeps ----
            h3 = []
            for l in range(3):
                ot = WIDTHS[l] // 128
                ndr, nbf = NDRP[l], NBF[l]
                kw = (1 + nbf) * 128      # wb cols per i-tile
                scl = sc[l]
                hacc = [None] * ot
                for q in range(NQ):
                    for o in range(ot):
                        row = (o * NQ + q) * 128
                        wbs = wbp.tile([128, KQ * 7 * 128], BF16, tag="wb",
                                       name=f"wb{l}_{q}_{o}")
                        nc.sync.dma_start(wbs[:, :WBCOLS[l]],
                                          wb_d[l][row:row + 128, :])
                        if ndr:
                            wqs = wqp.tile([128, KQ * 3 * 2 * 128], F8,
                                           tag="wq", name=f"wq{l}_{q}_{o}")
                            nc.sync.dma_start(wqs[:, :WQCOLS[l]],
                                              wq_d[l][row:row + 128, :])
                        acc = psA.tile([128, B], F32, tag="acc")
                        idx = 0
                        last = KQ * (1 + nbf + ndr) - 1
                        for kk in range(KQ):
                            i = q * KQ + kk
                            nc.tensor.matmul(
                                acc, wbs[:, kk * kw:kk * kw + 128],
                                silu_t[(l, i)], start=(idx == 0),
                                stop=(idx == last))
                            idx += 1
                            for m in range(nbf):
                                o0 = kk * kw + (1 + m) * 128
                                pr, g = divmod(ndr * 2 + m, 2)
                                nc.tensor.matmul(
                                    acc, wbs[:, o0:o0 + 128],
                                    pair_t[(l, i, pr)][:, g, :],
                                    start=False, stop=(idx == last))
                                idx += 1
                            for pp in range(ndr):
                                o0 = (kk * ndr + pp) * 256
                                lhs = wqs[:, o0:o0 + 256].rearrange(
                                    "p (two m) -> p two m", two=2)
                                nc.tensor.matmul(
                                    acc, lhs, pair_t[(l, i, pp)][:, :, :],
                                    start=False, stop=(idx == last),
                                    perf_mode=DR)
                                idx += 1
                        if q == 0:
                            hacc[o] = hap.tile([128, B], F16, tag="ha",
                                               name=f"ha{l}_{o}")
                            nc.scalar.activation(hacc[o], acc, AF.Copy,
                                                 scale=scl)
                        elif q < NQ - 1 or l < 2:
                            nc.vector.scalar_tensor_tensor(
                                hacc[o], acc, scl, hacc[o], OP.mult, OP.add)
                        else:
                            h3t = xsp.tile([128, B], BF16, tag="xs",
                                           name=f"h3_{o}")
                            nc.vector.scalar_tensor_tensor(
                                h3t, acc, scl, hacc[o], OP.mult, OP.add)
                            h3.append(h3t)
                            # interleave head matmuls into the final sweep
                            if o == 0:
                                acch = psH.tile([128, B], F32, tag="acch")
                            nc.tensor.matmul(acch[0:2, :],
                                             wh_sb[:, 2 * o:2 * o + 2],
                                             h3t, start=(o == 0),
                                             stop=(o == ot - 1))
                        # weave bases one quarter ahead; at q3 batch the next
                        # layer's silus and its quarter-0 bases
                        if q < NQ - 1 and o < KQ:
                            emit_unit(l, KQ * (q + 1) + o,
                                      h_cur[KQ * (q + 1) + o])
                        if q == NQ - 1 and l < 2:
                            emit_silu(l + 1, o, hacc[o])
                            if o < KQ:
                                emit_unit(l + 1, o, hacc[o])
                h_cur = hacc

            # ---- heads ----
            res = consts.tile([2, B], F32, tag="res")
            nc.vector.tensor_scalar(res, acch[0:2, :], hb_sb[:, 0:1], None,
                                    OP.add)
            nc.sync.dma_start(out[:], res)

    return _patch_json(nc)


# ---------------- host-side prep ----------------

_f32 = np.float32
_bf16 = ml_dtypes.bfloat16
_f8 = ml_dtypes.float8_e4m3


def _bases_np(h):
    """Closed-form b4 = 4*bases, numpy float32 (for the Gram matrix)."""
    out = np.empty(h.shape + (COEFF,), _f32)
    for c in range(COEFF):
        u = np.abs(h * _f32(SCL_IN) + _f32(-KNOT[c + 2] * SCL_IN))
        v = np.minimum(u + _f32(A2), 0.0)
        w = np.minimum(u + _f32(A1), 0.0)
        out[..., c] = (2.0 * w) ** 2 * w - v ** 3
    return out


def _gram():
    hs = (np.random.default_rng(7).standard_normal(100000) * 1.2).astype(_f32)
    b = _bases_np(hs)
    return (b.T @ b / len(b)).astype(_f32)


def _ls_q8(w_scaled, M):
    """Greedy fp8 quantization of (..., n) weight vectors minimizing the
    quadratic form with basis Gram M. Returns fp8-representable float32."""
    sh = w_scaled.shape
    ncf = sh[-1]
    W = w_scaled.reshape(-1, ncf).astype(_f32)
    Q = W.astype(_f8).astype(_f32)
    big_up = np.array(1000.0, _f8)
    big_dn = np.array(-1000.0, _f8)
    for _ in range(2):
        for c in range(ncf):
            g = (Q - W) @ M[:, c]
            w8 = Q[:, c].astype(_f8)
            stepped = np.where(g > 0, np.nextafter(w8, big_dn),
                               np.nextafter(w8, big_up)).astype(_f32)
            dq = stepped - Q[:, c]
            dcost = 2 * dq * g + dq * dq * M[c, c]
            take = dcost < 0
            Q[:, c] = np.where(take, stepped, Q[:, c])
    return Q.reshape(sh)


def _prep(inputs):
    feed = {}
    mw = np.asarray(inputs["mult_w"], _f32)
    feed["wm"] = np.ascontiguousarray(
        mw.reshape(32, 128, IT, 128).transpose(0, 3, 2, 1)
        .reshape(32 * 128, D)).astype(_bf16)
    mb = np.asarray(inputs["mult_b"], _f32)
    feed["mbg"] = np.ascontiguousarray(mb[:D].reshape(16, 128).T).astype(_f32)
    feed["mbv"] = np.ascontiguousarray(mb[D:].reshape(16, 128).T).astype(_f32)

    M = _gram()
    js = []
    for l, fo in enumerate(WIDTHS):
        fi = ([D] + WIDTHS)[l]
        ot, itl = fo // 128, fi // 128
        ndr, nbf = NDRP[l], NBF[l]
        npl = NPLAIN[l]
        sw = (np.asarray(inputs[f"spline_w{l}"], _f32)
              * np.asarray(inputs[f"scaler{l}"], _f32)[..., None]) / 4.0
        j = int(np.floor(np.log2(224.0 / np.abs(sw).max())))
        js.append(j)
        s = _f32(2.0 ** j)

        # bf16 strip: per i-tile [base | bf16 spline coeffs npl..5]
        bw = np.asarray(inputs[f"base_w{l}"], _f32) * s
        bwt = bw.reshape(ot, 128, itl, 128)              # [o, oc, it, p]
        cols = np.empty((ot, 128, itl, 128, 1 + nbf), _f32)
        cols[..., 0] = bwt
        if nbf:
            swt = (sw * s).reshape(ot, 128, itl, 128, COEFF)
            cols[..., 1:] = swt[..., npl:]
        arr = (cols.reshape(ot, 128, NQ, KQ, 128, 1 + nbf)
               .transpose(0, 2, 4, 3, 5, 1)              # [o,q,p,kk,m,oc]
               .reshape(ot * NQ * 128, WBCOLS[l]))
        feed[f"wb{l}"] = np.ascontiguousarray(arr).astype(_bf16)

        if ndr:
            swq = _ls_q8(sw[..., :npl] * s, M[:npl, :npl])
            swt = swq.reshape(ot, 128, itl, 128, npl)    # [o, oc, it, p, c]
            arr = (swt.reshape(ot, 128, NQ, KQ, 128, ndr, 2)
                   .transpose(0, 2, 4, 3, 5, 6, 1)       # [o,q,p,kk,pp,g,oc]
                   .reshape(ot * NQ * 128, WQCOLS[l]))
            feed[f"wq{l}"] = np.ascontiguousarray(arr).astype(_f8)

    whh = np.stack([np.asarray(inputs["reg_w"], _f32)[0],
                    np.asarray(inputs["aux_w"], _f32)[0]], axis=1)  # [1024,2]
    feed["wh"] = np.ascontiguousarray(
        whh.reshape(8, 128, 2).transpose(1, 0, 2).reshape(128, 16)
    ).astype(_bf16)
    feed["hb"] = np.array([[np.asarray(inputs["reg_b"], _f32)[0]],
                           [np.asarray(inputs["aux_b"], _f32)[0]]], _f32)
    return feed, tuple(js)


_NC = {}


def kernel(**inputs):
    from concourse.bass_utils import run_bass_kernel_spmd

    shared, js = _prep(inputs)
    if js not in _NC:
        _NC[js] = build(js)
    x_full = np.asarray(inputs["x"], np.float32)
    per_core = []
    for c in range(N_CORES):
        m = dict(shared)
        m["xT"] = np.ascontiguousarray(
            x_full[c * B:(c + 1) * B].T).astype(_bf16)
        per_core.append(m)
    res = run_bass_kernel_spmd(_NC[js], per_core, core_ids=list(range(N_CORES)))
    reg = np.concatenate([res.results[c]["out"][0] for c in range(N_CORES)])
    aux = np.concatenate([res.results[c]["out"][1] for c in range(N_CORES)])
    kernel.last_results = res
    return reg, aux
